# revision 21
# baseline (speedup 1.0000x reference)
"""BatchHardTripletLoss on 8 trn2 NeuronCores (Bass/Tile, SPMD data-parallel).

v4 design (fp8 DoubleRow Gram + label-sorted sparse masking + wide ACT
drains + DVE bf16 max-tree):

Host: rows are sorted by label, L2-normalized, scaled by S=16 and quantized to
fp8e4m3.  Each core owns 512 consecutive sorted anchor rows and computes the
[512, 4096] block of the (scaled) Gram matrix  S^2 * (e_i . e_j)  with fp8
DoubleRow matmuls (K=256 per instruction, 2 per 128x512 sub-block).

Label masking: because rows are sorted, all same-label (positive) pairs of a
core's rows live in the core's own column chunk plus its sorted neighbors.
The host permutes column chunks per core so those sit at positions 0..2; a
single K<=128 one-hot matmul per (row-tile, mask-chunk) adds
-4*S^2*[l_i == l_j] there.  Mask matmuls are emitted only for (m, n) blocks
where some core has a shared class (union across cores; zero one-hots are
harmless for the others).

Pipeline per 128-row tile m: 4 two-bank PSUM tiles [128, 1024] are filled by
2x2 DR matmul groups, drained by single wide ACT copies into a bf16 strip
g[m] [128, 4096].  DVE then runs a tensor_tensor max tree (bf16 2x mode):
t1a=max(g0,g1), t1b=max(g2,g3), t2=max(t1a,t1b), ttr(t2 halves)->rmx.
The min scan (hardest positive) only covers the <=3 chunks that can contain
positives (union across cores): a chained ttr/tt over those strips -> rmn.
Unshifted entries are >= -S^2, far above any shifted positive, so extra
chunks in the min scan are harmless.

Tail (merged across m): loss = relu(rmx - rmn + (margin-4)*S^2) * valid,
summed by a ones-matmul; host divides by S^2 * n_valid.  Validity depends
only on labels and is computed host-side.
"""

import os
from contextlib import ExitStack

import numpy as np
import ml_dtypes

import concourse.bass as bass
import concourse.bacc as bacc
import concourse.mybir as mybir
import concourse.tile as tile
from concourse.bass_utils import run_bass_kernel_spmd

F32 = mybir.dt.float32
BF16 = mybir.dt.bfloat16
FP8 = mybir.dt.float8e4
AF = mybir.ActivationFunctionType
ALU = mybir.AluOpType
AX = mybir.AxisListType
DR = mybir.MatmulPerfMode.DoubleRow
FP8NP = ml_dtypes.float8_e4m3

B, D, C = 4096, 512, 512
NCORES = 8
RPC = B // NCORES            # rows per core = 512
NCH = 512                    # column chunk size (PSUM bank = 512 fp32)
NM = RPC // 128              # 128-row tiles per core = 4
NN = B // NCH                # column chunks = 8
KD = D // 128                # contraction k-subtiles = 4
S = 16.0                     # fp8 quantization scale
S2 = S * S
MARGIN = 0.2
BIG = 4.0


def build_program(nmask, mask_blocks, min_chunks):
    """mask_blocks: frozenset of (m, n) needing a one-hot mask matmul.
    min_chunks: tuple over m of tuple of chunk positions the min must scan."""
    nc = bacc.Bacc("TRN2", target_bir_lowering=False, debug=False)
    ET_d = nc.declare_dram_parameter("ET", [128, NN * KD * NCH], FP8, isOutput=False)
    # masks: MP ([128, NM*128]) and MN ([128, NM*nmask*NCH]) concatenated
    MKW = NM * 128 + NM * nmask * NCH
    MK_d = nc.declare_dram_parameter("MASKS", [128, MKW], FP8, isOutput=False)
    val_d = nc.declare_dram_parameter("valid", [128, NM], F32, isOutput=False)
    out_d = nc.declare_dram_parameter("out", [1, NM], F32, isOutput=True)

    with tile.TileContext(nc) as tc, ExitStack() as ctx:
        const = ctx.enter_context(tc.tile_pool(name="const", bufs=1))
        bigp = ctx.enter_context(tc.tile_pool(name="bigp", bufs=1))
        gp = ctx.enter_context(tc.tile_pool(name="gp", bufs=1))
        scr = ctx.enter_context(tc.tile_pool(name="scr", bufs=2))
        sm = ctx.enter_context(tc.tile_pool(name="small", bufs=1))
        psM = ctx.enter_context(tc.tile_pool(name="psM", bufs=2, space="PSUM"))

        # input tiles
        et = bigp.tile([128, NN, KD, NCH], FP8, tag="et")
        mk = sm.tile([128, MKW], FP8, tag="mk")

        def mp_ap(m):
            return mk[:, m * 128:(m + 1) * 128]

        def mnt_ap(m, n):
            o = NM * 128 + (m * nmask + n) * NCH
            return mk[:, o:o + NCH]

        # DMA issue is serialized per engine queue (~0.6us each), so spread
        # the loads across idle engine queues and put first-needed first.
        # Chunk n of ET is contiguous [128, KD*NCH] in DRAM (chunk-major).
        CW = KD * NCH
        nc.scalar.dma_start(et[:, 0, 0:2, :], ET_d[:, 0:CW // 2])
        nc.gpsimd.dma_start(et[:, 0, 2:4, :], ET_d[:, CW // 2:CW])
        nc.sync.dma_start(et[:, 1, :, :], ET_d[:, CW:2 * CW])
        nc.gpsimd.dma_start(mk[:, :], MK_d[:, :])
        nc.sync.dma_start(et[:, 2:4, :, :], ET_d[:, 2 * CW:4 * CW])
        nc.scalar.dma_start(et[:, 4:6, :, :], ET_d[:, 4 * CW:6 * CW])
        nc.sync.dma_start(et[:, 6:8, :, :], ET_d[:, 6 * CW:8 * CW])

        # constants
        relu_bias = const.tile([128, 1], F32, tag="rbias")
        nc.vector.memset(relu_bias[:], (MARGIN - BIG) * S2)
        ones_cf = const.tile([128, 1], F32, tag="ones")
        nc.vector.memset(ones_cf[:], 1.0)
        val_t = const.tile([128, NM], F32, tag="val")
        nc.gpsimd.dma_start(val_t[:], val_d[:, :])

        g = [
            gp.tile([128, B], BF16, tag=f"g{m}", name=f"g{m}") for m in range(NM)
        ]
        rmn = sm.tile([128, NM], F32, tag="rmn")
        rmx = sm.tile([128, NM], F32, tag="rmx")

        # PE warmup: dummy DR matmuls on a memset tile keep PE busy (and its
        # clock ramping) while the first ET chunk DMA is in flight.
        wrm = const.tile([128, 2, NCH], FP8, tag="wrm")
        nc.gpsimd.memset(wrm[:], 0.0)
        wps = psM.tile([128, 4 * NCH], F32, tag="ps", name="warmup_ps")
        for w in range(4):
            nc.tensor.matmul(
                wps[:, 0:NCH], lhsT=wrm[:, :, 0:128], rhs=wrm[:, :, :],
                start=True, stop=True, perf_mode=DR,
            )

        GW = 4 * NCH  # drain-group width (one 4-bank psum tile)

        def emit_group(m, j):
            ps = psM.tile([128, GW], F32, tag="ps", name=f"ps{m}_{j}")
            for h in range(4):
                n = 4 * j + h
                has_mask = (m, n) in mask_blocks
                dst = ps[:, h * NCH:(h + 1) * NCH]
                for kp in range(KD // 2):
                    nc.tensor.matmul(
                        dst,
                        lhsT=et[:, 0, 2 * kp:2 * kp + 2, m * 128:(m + 1) * 128],
                        rhs=et[:, n, 2 * kp:2 * kp + 2, :],
                        start=(kp == 0),
                        stop=(kp == KD // 2 - 1 and not has_mask),
                        perf_mode=DR,
                    )
                if has_mask:
                    nc.tensor.matmul(
                        dst,
                        lhsT=mp_ap(m),
                        rhs=mnt_ap(m, n),
                        start=False,
                        stop=True,
                    )
            nc.scalar.copy(g[m][:, j * GW:(j + 1) * GW], ps[:])

        def emit_red_hi(m):
            # second-half tree level on GPSIMD (otherwise idle), rest on DVE
            t1b = scr.tile([128, 2 * NCH], BF16, tag="t1b", name=f"t1b{m}")
            nc.vector.tensor_tensor(
                t1b[:], g[m][:, 4 * NCH:6 * NCH], g[m][:, 6 * NCH:8 * NCH], ALU.max
            )
            t1a = t1a_of[m]
            t2 = scr.tile([128, 2 * NCH], BF16, tag="t2", name=f"t2{m}")
            nc.vector.tensor_tensor(t2[:], t1a[:], t1b[:], ALU.max)
            t3 = scr.tile([128, NCH], BF16, tag="t3", name=f"t3{m}")
            nc.vector.tensor_tensor(
                t3[:], t2[:, 0:NCH], t2[:, NCH:2 * NCH], ALU.max
            )
            nc.vector.tensor_reduce(rmx[:, m:m + 1], t3[:], AX.X, ALU.max)

        # pipeline: group (m, 0) -> red_lo(m) while group (m, 1) fills
        t1a_of = {}
        for m in range(NM):
            emit_group(m, 0)
            # stash t1a handle via emit_red_lo (records into t1a_of)
            t1a = scr.tile([128, 2 * NCH], BF16, tag="t1a", name=f"t1a{m}")
            nc.vector.tensor_tensor(
                t1a[:], g[m][:, 0:2 * NCH], g[m][:, 2 * NCH:4 * NCH], ALU.max
            )
            t1a_of[m] = t1a
            mc = min_chunks[m]
            lo, hi = min(mc), max(mc)
            span = hi - lo + 1
            if span == 1:
                nc.vector.tensor_reduce(
                    rmn[:, m:m + 1], g[m][:, lo * NCH:(lo + 1) * NCH],
                    AX.X, ALU.min,
                )
            else:
                x0 = scr.tile([128, NCH], BF16, tag="x0", name=f"x0{m}")
                nc.vector.tensor_tensor(
                    x0[:],
                    g[m][:, lo * NCH:(lo + 1) * NCH],
                    g[m][:, (lo + 1) * NCH:(lo + 2) * NCH],
                    ALU.min,
                )
                for e in range(2, span):
                    nc.vector.tensor_tensor(
                        x0[:], x0[:],
                        g[m][:, (lo + e) * NCH:(lo + e + 1) * NCH],
                        ALU.min,
                    )
                nc.vector.tensor_reduce(
                    rmn[:, m:m + 1], x0[:], AX.X, ALU.min
                )
            emit_group(m, 1)
            emit_red_hi(m)

        # merged tail: loss = relu(rmx - rmn + (margin-4)*S2) * valid
        dlt = sm.tile([128, NM], F32, tag="dlt")
        nc.vector.tensor_tensor(dlt[:], rmx[:, :], rmn[:, :], ALU.subtract)
        rl = sm.tile([128, NM], F32, tag="rl")
        nc.scalar.activation(rl[:], dlt[:], AF.Relu, bias=relu_bias[:])
        loss_all = sm.tile([128, NM], F32, tag="loss")
        nc.vector.tensor_tensor(loss_all[:], rl[:], val_t[:, :], ALU.mult)

        out_ps = psM.tile([128, 2 * NCH], F32, tag="ps", name="out_ps")
        nc.tensor.matmul(
            out_ps[0:1, 0:NM], lhsT=ones_cf[:], rhs=loss_all[:, :],
            start=True, stop=True,
        )
        out_sb = sm.tile([1, NM], F32, tag="outsb")
        nc.vector.tensor_copy(out_sb[:], out_ps[0:1, 0:NM])
        nc.sync.dma_start(out_d[:, :], out_sb[:])

    nc.compile()
    return nc


def host_prepare(embeddings, labels):
    """Sort by label, normalize+quantize, build per-core layouts and masks."""
    E = np.asarray(embeddings, dtype=np.float32)
    lab = np.asarray(labels).astype(np.int64)
    order0 = np.argsort(lab, kind="stable")
    ls = lab[order0]
    Es = E[order0]
    nrm = np.maximum(np.linalg.norm(Es, axis=1, keepdims=True), 1e-12)
    En = Es / nrm
    Q8 = (S * En).astype(FP8NP)                  # [B, D]
    QT = np.ascontiguousarray(Q8.T)              # [D, B]

    cnt = np.bincount(ls, minlength=int(ls.max()) + 1)[ls]
    valid_s = ((cnt >= 2) & (cnt <= B - 1)).astype(np.float32)
    n_valid = max(int(valid_s.sum()), 1)

    orders, needs = [], []
    for c in range(NCORES):
        rows = ls[c * RPC:(c + 1) * RPC]
        lo = int(np.searchsorted(ls, rows[0], side="left"))
        hi = int(np.searchsorted(ls, rows[-1], side="right"))
        need = list(range(lo // NCH, (hi - 1) // NCH + 1))
        order = (
            [c]
            + ([c - 1] if c > 0 else [])
            + ([c + 1] if c < NN - 1 else [])
        )
        order += [n for n in need if n not in order]
        order += [n for n in range(NN) if n not in order]
        orders.append(order)
        needs.append(need)
    nmask = max(3, max(len(n) for n in needs))

    # structural info shared by all cores (program is SPMD-shared):
    # which (m, n) blocks need a mask matmul, and which chunk positions the
    # min scan must cover per m -- union across cores.
    mask_blocks = set()
    min_chunks = [set() for _ in range(NM)]
    in_maps = []
    for c in range(NCORES):
        order = orders[c]
        pos_of = {n: i for i, n in enumerate(order)}
        rows = ls[c * RPC:(c + 1) * RPC]
        ETc = np.empty((128, NN, KD, NCH), dtype=FP8NP)
        for pos, n in enumerate(order):
            blk = QT[:, n * NCH:(n + 1) * NCH]   # [D, NCH]
            ETc[:, pos] = blk.reshape(KD, 128, NCH).transpose(1, 0, 2)
        MP = np.zeros((128, NM, 128), dtype=FP8NP)
        MN = np.zeros((128, NM, nmask, NCH), dtype=FP8NP)
        for m in range(NM):
            rl = rows[m * 128:(m + 1) * 128]
            cm, w_inv = np.unique(rl, return_inverse=True)
            MP[w_inv, m, np.arange(128)] = FP8NP(32.0)
            # columns of this tile's classes (sorted => contiguous range)
            lo = int(np.searchsorted(ls, rl[0], side="left"))
            hi = int(np.searchsorted(ls, rl[-1], side="right"))
            for n in range(lo // NCH, (hi - 1) // NCH + 1):
                pos = pos_of[n]
                assert pos < nmask, (c, m, n, pos, order)
                colsn = ls[n * NCH:(n + 1) * NCH]
                match = cm[:, None] == colsn[None, :]
                MN[:len(cm), m, pos, :][match] = FP8NP(-32.0)
                mask_blocks.add((m, pos))
                min_chunks[m].add(pos)
        vmat = np.ascontiguousarray(
            valid_s[c * RPC:(c + 1) * RPC].reshape(NM, 128).T
        )
        masks = np.concatenate(
            [MP.reshape(128, NM * 128), MN.reshape(128, NM * nmask * NCH)],
            axis=1,
        )
        in_maps.append(
            {
                "ET": np.ascontiguousarray(ETc.reshape(128, NN * KD * NCH)),
                "MASKS": np.ascontiguousarray(masks),
                "valid": vmat,
            }
        )
    struct = (
        nmask,
        frozenset(mask_blocks),
        tuple(tuple(sorted(min_chunks[m])) for m in range(NM)),
    )
    return in_maps, n_valid, struct


_prog_cache = {}


def _get_program(struct):
    if struct not in _prog_cache:
        _prog_cache[struct] = build_program(*struct)
    return _prog_cache[struct]


LAST_RESULT = None


def kernel(embeddings, labels):
    global LAST_RESULT
    in_maps, n_valid, struct = host_prepare(embeddings, labels)
    nc = _get_program(struct)
    trace = bool(int(os.environ.get("TRIPLET_TRACE", "0")))
    res = run_bass_kernel_spmd(nc, in_maps, list(range(NCORES)), trace=trace)
    LAST_RESULT = res
    loss_sum = float(sum(r["out"].astype(np.float64).sum() for r in res.results))
    return np.array(loss_sum / (S2 * n_valid), dtype=np.float32)
